# revision 1
# baseline (speedup 1.0000x reference)
"""Attentional Factorization Machine — Trainium2 Bass kernel (8 NeuronCores).

Sharding: data-parallel over batch (2048/8 = 256 per core, as 2 groups of 128).

Per-core dataflow (per 128-batch group, split as two 64-batch halves packed on
SBUF partition halves):
  1. Indirect-DMA gather of an augmented table [500000, 65] (= emb ++ lin_w)
     into batch-major rows [128, 50*65].
  2. PE transposes per field -> factors^T layout [128 part = d(x2 halves),
     50*64 cols = (field, batch)].
  3. Pairwise products inter^T formed on DVE per pair-block i (f_i broadcast
     against f_j, j>i) -> [128, (49-i)*64].
  4. PE matmul1 (stationary W1*diag(|w2|), K=64, two row/col-group tiles) ->
     h'^T in PSUM; ACT/DVE drain with fused bias+ReLU -> hs.
  5. PE M=1 matmuls: logits = sign(w2)^T hs, intersum = ones^T inter^T
     (4 tile-position-packed strips per chunk); strips drained via DVE/ACT,
     compacted to batch-major [128, 1225] via SBUF->SBUF DMA.
  6. Softmax over pairs on ACT/DVE (exp with fused free-axis accumulation),
     attended = (intersum . scores), plus linear term from the gathered
     lin_w column -> y [128, 1].
"""

import sys

for _p in ("/opt/trn_rl_repo",):
    if _p not in sys.path:
        sys.path.insert(0, _p)

import numpy as np

import concourse.bass as bass
from concourse import bacc
import concourse.mybir as mybir
from concourse.tile import TileContext
from concourse.masks import make_identity
from concourse.bass_utils import run_bass_kernel_spmd

F = 50
D = 64
CARD = 10000
B = 2048
NCORES = 8
BPC = B // NCORES          # 256 batches per core
G = 2                      # groups of 128 per core
P = F * (F - 1) // 2       # 1225 pairs
ROWD = D + 1               # augmented row width (emb ++ lin_w)
FP32 = mybir.dt.float32


def _off(i):
    """Start col (per batch) of pair-block i in pair-index space."""
    return i * (F - 1) - i * (i - 1) // 2  # sum_{k<i} (F-1-k)


def build_nc():
    nc = bacc.Bacc(None, target_bir_lowering=False)

    idx_d = nc.dram_tensor("idx", [BPC, F], mybir.dt.int32, kind="ExternalInput")
    tab_d = nc.dram_tensor("tab", [CARD * F, ROWD], FP32, kind="ExternalInput")
    w1_d = nc.dram_tensor("w1", [D, D], FP32, kind="ExternalInput")
    w2c_d = nc.dram_tensor("w2c", [D, 1], FP32, kind="ExternalInput")
    b1c_d = nc.dram_tensor("b1c", [D, 1], FP32, kind="ExternalInput")
    linb_d = nc.dram_tensor("linb", [128, 1], FP32, kind="ExternalInput")
    y_d = nc.dram_tensor("y", [BPC, 1], FP32, kind="ExternalOutput")
    dbg_gt = nc.dram_tensor("dbg_gt", [128, 130], FP32, kind="ExternalOutput")
    dbg_fact = nc.dram_tensor("dbg_fact", [128, 256], FP32, kind="ExternalOutput")
    dbg_lgt = nc.dram_tensor("dbg_lgt", [128, 98], FP32, kind="ExternalOutput")
    dbg_one = nc.dram_tensor("dbg_one", [128, 98], FP32, kind="ExternalOutput")

    with TileContext(nc) as tc:
        with (
            tc.tile_pool(name="const", bufs=1) as cpool,
            tc.tile_pool(name="gath", bufs=2) as gpool,
            tc.tile_pool(name="fact", bufs=2) as fpool,
            tc.tile_pool(name="inter", bufs=2) as ipool,
            tc.tile_pool(name="slab", bufs=2) as slpool,
            tc.tile_pool(name="hs", bufs=4) as hpool,
            tc.tile_pool(name="bm", bufs=2) as bmpool,
            tc.tile_pool(name="small", bufs=4) as smpool,
            tc.tile_pool(name="ptp", bufs=2, space="PSUM") as ptp,
            tc.tile_pool(name="php", bufs=2, space="PSUM") as php,
            tc.tile_pool(name="psp", bufs=2, space="PSUM") as psp,
        ):
            # ---------------- constants / weights prep ----------------
            ident = cpool.tile([128, 128], FP32)
            make_identity(nc, ident[:])
            # dummy transpose: syncs PE with identity's producer once, so real
            # transposes carry a single wait (transpose matmuls allow only one)
            warm = ptp.tile([64, 64], FP32, tag="tpa")
            nc.tensor.transpose(warm[:], ident[0:64, 0:64], ident[0:64, 0:64])

            idx_sb = cpool.tile([128, G * F], mybir.dt.int32)
            nc.sync.dma_start(
                out=idx_sb[:].rearrange("p (g f) -> p g f", g=G),
                in_=idx_d[:].rearrange("(g p) f -> p g f", g=G),
            )

            # stationaries, replicated on both partition halves
            w1s = cpool.tile([128, D], FP32)
            nc.sync.dma_start(out=w1s[0:64, :], in_=w1_d[:])
            nc.sync.dma_start(out=w1s[64:128, :], in_=w1s[0:64, :])

            w2big = cpool.tile([128, 1], FP32)
            nc.sync.dma_start(out=w2big[0:64, :], in_=w2c_d[:])
            nc.sync.dma_start(out=w2big[64:128, :], in_=w2big[0:64, :])

            b1s = cpool.tile([128, 1], FP32)
            nc.sync.dma_start(out=b1s[0:64, :], in_=b1c_d[:])
            nc.sync.dma_start(out=b1s[64:128, :], in_=b1s[0:64, :])

            ones_c = cpool.tile([128, 1], FP32)
            nc.vector.memset(ones_c[:], 1.0)

            linb = cpool.tile([128, 1], FP32)
            nc.sync.dma_start(out=linb[:], in_=linb_d[:])

            # ---------------- main loop over 128-batch groups ----------------
            for g in range(G):
                gt = gpool.tile([128, F * ROWD], FP32, tag="gt")
                for f in range(F):
                    nc.gpsimd.indirect_dma_start(
                        out=gt[:, f * ROWD:(f + 1) * ROWD],
                        out_offset=None,
                        in_=tab_d[:],
                        in_offset=bass.IndirectOffsetOnAxis(
                            ap=idx_sb[:, g * F + f:g * F + f + 1], axis=0
                        ),
                    )

                # linear term: sum over fields of the gathered lin_w column
                gt3 = gt[:].rearrange("p (f e) -> p f e", e=ROWD)
                lin_g = smpool.tile([128, 1], FP32, tag="lin")
                nc.vector.tensor_reduce(
                    out=lin_g[:], in_=gt3[:, :, D:ROWD].rearrange("p f e -> p (f e)"),
                    axis=mybir.AxisListType.X, op=mybir.AluOpType.add,
                )
                lin_t = smpool.tile([128, 1], FP32, tag="lint")
                nc.vector.tensor_tensor(
                    out=lin_t[:], in0=lin_g[:], in1=linb[:], op=mybir.AluOpType.add
                )

                # factors^T: [d (x2 halves), (field, batch64)]
                fact = fpool.tile([128, F * D], FP32, tag="fact")
                tmpb = fpool.tile([64, F * D], FP32, tag="tmpb")
                for fb in range(0, F, 8):
                    nf = min(8, F - fb)
                    tpa = ptp.tile([64, 8, D], FP32, tag="tpa")
                    tpb = ptp.tile([64, 8, D], FP32, tag="tpb")
                    for j in range(nf):
                        f = fb + j
                        nc.tensor.transpose(
                            tpa[:, j, :],
                            gt[0:64, f * ROWD:f * ROWD + D],
                            ident[0:64, 0:64],
                        )
                        nc.tensor.transpose(
                            tpb[:, j, :],
                            gt[64:128, f * ROWD:f * ROWD + D],
                            ident[64:128, 64:128],
                        )
                    nc.scalar.activation(
                        out=fact[0:64, fb * D:(fb + nf) * D],
                        in_=tpa[:, 0:nf, :],
                        func=mybir.ActivationFunctionType.Copy,
                    )
                    nc.vector.tensor_copy(
                        tmpb[:, fb * D:(fb + nf) * D], tpb[:, 0:nf, :]
                    )
                nc.sync.dma_start(out=fact[64:128, :], in_=tmpb[:])

                lgt_bm = bmpool.tile([128, P], FP32, tag="lgt")
                one_bm = bmpool.tile([128, P], FP32, tag="one")

                # pair blocks
                for i in range(F - 1):
                    W = F - 1 - i
                    blk = W * D
                    inter = ipool.tile([128, blk], FP32, tag="inter")
                    nc.vector.tensor_tensor(
                        out=inter[:].rearrange("p (b j) -> p b j", j=W),
                        in0=fact[:, i * D:(i + 1) * D]
                        .rearrange("p (b o) -> p b o", o=1)
                        .to_broadcast([128, D, W]),
                        in1=fact[:, (i + 1) * D:F * D]
                        .rearrange("p (j b) -> p b j", b=D),
                        op=mybir.AluOpType.mult,
                    )

                    slab = slpool.tile([128, blk], FP32, tag="slab")
                    nchunk = 0
                    for c in range(0, blk, 512):
                        N = min(512, blk - c)
                        hp = php.tile([128, 512], FP32, tag="hp")
                        nc.tensor.matmul(
                            hp[0:64, 0:N], w1s[0:64, :], inter[0:64, c:c + N],
                            start=True, stop=True,
                        )
                        nc.tensor.matmul(
                            hp[64:128, 0:N], w1s[64:128, :], inter[64:128, c:c + N],
                            start=True, stop=True,
                        )
                        hs = hpool.tile([128, 512], FP32, tag="hs")
                        if nchunk % 2 == 0:
                            nc.scalar.activation(
                                out=hs[:, 0:N], in_=hp[:, 0:N],
                                func=mybir.ActivationFunctionType.Relu,
                                bias=b1s[:, 0:1],
                            )
                        else:
                            nc.vector.tensor_scalar(
                                out=hs[:, 0:N], in0=hp[:, 0:N],
                                scalar1=b1s[:, 0:1], scalar2=0.0,
                                op0=mybir.AluOpType.add, op1=mybir.AluOpType.max,
                            )
                        sp = psp.tile([128, 512], FP32, tag="sp")
                        nc.tensor.matmul(
                            sp[0:1, 0:N], w2big[0:64, 0:1], hs[0:64, 0:N],
                            start=True, stop=True,
                        )
                        nc.tensor.matmul(
                            sp[32:33, 0:N], w2big[64:128, 0:1], hs[64:128, 0:N],
                            start=True, stop=True, tile_position=(64, 32),
                        )
                        nc.tensor.matmul(
                            sp[64:65, 0:N], ones_c[0:64, 0:1], inter[0:64, c:c + N],
                            start=True, stop=True, tile_position=(0, 64),
                        )
                        nc.tensor.matmul(
                            sp[96:97, 0:N], ones_c[64:128, 0:1], inter[64:128, c:c + N],
                            start=True, stop=True, tile_position=(64, 96),
                        )
                        if nchunk % 2 == 0:
                            nc.vector.tensor_copy(slab[:, c:c + N], sp[:, 0:N])
                        else:
                            nc.scalar.activation(
                                out=slab[:, c:c + N], in_=sp[:, 0:N],
                                func=mybir.ActivationFunctionType.Copy,
                            )
                        nchunk += 1

                    # compact strips to batch-major
                    o = _off(i)
                    s3 = slab[:].rearrange("p (b j) -> p b j", j=W)
                    nc.sync.dma_start(
                        out=lgt_bm[0:64, o:o + W], in_=s3[0:1, :, :]
                    )
                    nc.sync.dma_start(
                        out=lgt_bm[64:128, o:o + W], in_=s3[32:33, :, :]
                    )
                    nc.sync.dma_start(
                        out=one_bm[0:64, o:o + W], in_=s3[64:65, :, :]
                    )
                    nc.sync.dma_start(
                        out=one_bm[64:128, o:o + W], in_=s3[96:97, :, :]
                    )

                # softmax + attended + linear
                ex = bmpool.tile([128, P], FP32, tag="ex")
                zsum = smpool.tile([128, 1], FP32, tag="z")
                nc.scalar.activation(
                    out=ex[:], in_=lgt_bm[:],
                    func=mybir.ActivationFunctionType.Exp,
                    accum_out=zsum[:],
                )
                num = smpool.tile([128, 1], FP32, tag="num")
                wex = bmpool.tile([128, P], FP32, tag="wex")
                nc.vector.tensor_tensor(
                    out=wex[:], in0=ex[:], in1=one_bm[:], op=mybir.AluOpType.mult
                )
                nc.vector.tensor_reduce(
                    out=num[:], in_=wex[:],
                    axis=mybir.AxisListType.X, op=mybir.AluOpType.add,
                )
                rz = smpool.tile([128, 1], FP32, tag="rz")
                nc.vector.reciprocal(rz[:], zsum[:])
                att = smpool.tile([128, 1], FP32, tag="att")
                nc.vector.tensor_tensor(
                    out=att[:], in0=num[:], in1=rz[:], op=mybir.AluOpType.mult
                )
                yg = smpool.tile([128, 1], FP32, tag="yg")
                nc.vector.tensor_tensor(
                    out=yg[:], in0=att[:], in1=lin_t[:], op=mybir.AluOpType.add
                )
                nc.sync.dma_start(out=y_d[g * 128:(g + 1) * 128, :], in_=yg[:])
                if g == 0:
                    nc.sync.dma_start(out=dbg_gt[:], in_=gt[:, 0:130])
                    nc.sync.dma_start(out=dbg_fact[:], in_=fact[:, 0:256])
                    nc.sync.dma_start(out=dbg_lgt[:], in_=lgt_bm[:, 0:98])
                    nc.sync.dma_start(out=dbg_one[:], in_=one_bm[:, 0:98])

    nc.compile()
    return nc


_CACHE = {}


def kernel(x, emb, W1, b1, w2, b2, lin_w, lin_b):
    x = np.asarray(x)
    emb = np.asarray(emb, dtype=np.float32)
    W1 = np.asarray(W1, dtype=np.float32)
    b1 = np.asarray(b1, dtype=np.float32)
    w2 = np.asarray(w2, dtype=np.float32)
    lin_w = np.asarray(lin_w, dtype=np.float32)
    lin_b = np.asarray(lin_b, dtype=np.float32)

    # host-side input staging (layout only): global row ids + augmented table
    idx = (x.astype(np.int64) + (np.arange(F, dtype=np.int64) * CARD)[None, :])
    idx = idx.astype(np.int32)
    tab = np.concatenate([emb, lin_w.reshape(-1, 1)], axis=1).astype(np.float32)
    tab = np.ascontiguousarray(tab)
    w2c = np.ascontiguousarray(w2.reshape(D, 1))
    b1c = np.ascontiguousarray(b1.reshape(D, 1))
    linb = np.broadcast_to(
        lin_b.reshape(1, 1) + b2.reshape(1, 1) * 0.0, (128, 1)
    ).astype(np.float32).copy()

    if "nc" not in _CACHE:
        _CACHE["nc"] = build_nc()
    nc = _CACHE["nc"]

    in_maps = []
    for c in range(NCORES):
        in_maps.append({
            "idx": np.ascontiguousarray(idx[c * BPC:(c + 1) * BPC]),
            "tab": tab,
            "w1": W1,
            "w2c": w2c,
            "b1c": b1c,
            "linb": linb,
        })

    _CACHE["last_in_maps"] = in_maps
    res = run_bass_kernel_spmd(nc, in_maps, core_ids=list(range(NCORES)))
    outs = [res.results[c]["y"] for c in range(NCORES)]
    return np.concatenate(outs, axis=0).astype(np.float32)


if __name__ == "__main__":
    sys.path.insert(0, "/root/problem")
    import reference

    inputs = {k: np.asarray(v) for k, v in reference.setup_inputs().items()}
    y = kernel(**inputs)
    print(y.shape, y.dtype, y[:4, 0])

